# revision 14
# baseline (speedup 1.0000x reference)
"""Trainium2 Bass kernel for the KalmanFilter linear recurrence.

  x = data - mean;  z0 = R @ x[0];  drive = inputs @ C.T
  z_{t+1} = A z_t + drive[t]   (T = 32768 steps, dim 512)
  result  = Z[1:] @ B.T + mean

Strategy (8 NeuronCores, sequence-parallel, no collectives):
  - ||A^k|| decays like 0.9^k (spectral radius 0.9), so the recurrence
    forgets its state after H=128 steps to ~1e-5 relative.
  - Each core owns 4096 contiguous steps, split into 256 chunks of S=16
    steps + K=8 extra "halo" chunks covering the preceding H=128 steps.
  - Phase A: batched zero-init scan over all 268 chunks (state tiles
    [512, 268], 15 matmul steps) -> per-chunk accumulated drives b_c.
  - Phase B: chunk-start states w_c = sum_{p=0}^{K-1} (A^16)^p b_{c-1-p}
    (banded combine truncated at ||A^128|| ~ 1e-6 of a unit). The
    (A^16)^p factors are computed ON DEVICE by repeated squaring (f32r),
    so no big power-matrix upload is needed.
  - Phase C: re-scan the 256 real chunks from inits w_c; each step also
    applies the output projection B.T (fp16) and quantizes rows to int8
    with a per-row (per-timestep) scale = rowmax/127 computed on device.
  - z0 only affects output rows 0..H-1 (through A^n z0); that correction
    is added on the host, so the device never sees `data`/`R`.
  Wall time over the axon tunnel is transfer-bound (~40 MB/s), so all
  I/O is minimized: u is shipped transposed in fp16, B/C in fp16, A in
  f32, output as int8 + f32 row scales; mean is added on the host.
"""
import numpy as np
import concourse.bacc as bacc
import concourse.mybir as mybir
from concourse import tile
from concourse.bass_utils import run_bass_kernel_spmd

T = 32768
DZ = 512
DU = 256
NCORE = 8
TLOC = T // NCORE          # 4096
S = 16                     # steps per chunk
BCH = TLOC // S            # 256 chunks per core
H = 128                    # halo steps (forgetting horizon)
K = H // S                 # 8 banded taps (incl. identity)
NCH = BCH + K              # 268 chunks in phase A
ULEN = TLOC + H            # 4288 drive rows per core
UPAD = ((ULEN + 127) // 128) * 128   # padded to a multiple of 128
MAGIC = 12582912.0         # 1.5 * 2^23: float32 round-to-nearest-int trick

f32 = mybir.dt.float32
f32r = mybir.dt.float32r
fp16 = mybir.dt.float16
i8 = mybir.dt.int8

_CACHE = {}


def _emit(nc):
    ut_d = nc.dram_tensor("ut", (DU, UPAD), fp16, kind="ExternalInput")
    cb_d = nc.dram_tensor("cb", (6, 128, DZ), fp16, kind="ExternalInput")
    at_d = nc.dram_tensor("at", (DZ, DZ), f32r, kind="ExternalInput")
    id_d = nc.dram_tensor("id", (128, 128), f32r, kind="ExternalInput")
    outq_d = nc.dram_tensor("outq", (TLOC, DZ), i8, kind="ExternalOutput")
    osc_d = nc.dram_tensor("osc", (TLOC,), f32, kind="ExternalOutput")

    with tile.TileContext(nc) as tc:
        with tc.tile_pool(name="const", bufs=1) as cpool, \
             tc.tile_pool(name="dt", bufs=1) as dpool, \
             tc.tile_pool(name="ut", bufs=1) as utpool, \
             tc.tile_pool(name="pw", bufs=2) as wpool, \
             tc.tile_pool(name="st", bufs=2) as stpool, \
             tc.tile_pool(name="ob", bufs=3) as opool, \
             tc.tile_pool(name="ps", bufs=6, space="PSUM") as pp, \
             tc.tile_pool(name="pst", bufs=2, space="PSUM") as ppt:

            # ---- constant loads ----
            at_sb = [cpool.tile([128, DZ], f32r, tag=f"at{k}", name=f"at{k}") for k in range(4)]
            ct_sb = [cpool.tile([128, DZ], fp16, tag=f"ct{k}", name=f"ct{k}") for k in range(2)]
            bt_sb = [cpool.tile([128, DZ], fp16, tag=f"bt{k}", name=f"bt{k}") for k in range(4)]
            id_sb = cpool.tile([128, 128], f32r, tag="id")
            for k in range(4):
                nc.sync.dma_start(at_sb[k][:], at_d[128 * k:128 * (k + 1), :])
                nc.sync.dma_start(bt_sb[k][:], cb_d[2 + k])
            for k in range(2):
                nc.sync.dma_start(ct_sb[k][:], cb_d[k])
            nc.sync.dma_start(id_sb[:], id_d[:])

            ut_sb = [utpool.tile([128, UPAD], fp16, tag=f"ut{k}", name=f"ut{k}") for k in range(2)]
            for k in range(2):
                nc.sync.dma_start(ut_sb[k][:], ut_d[128 * k:128 * (k + 1), :])

            # drive rows (transposed): dT[m] holds drive.T[128m:128(m+1), :]
            dt_sb = [dpool.tile([128, UPAD], f32, tag=f"dt{m}", name=f"dt{m}") for m in range(4)]
            for nb in range((UPAD + 511) // 512):   # blocks of <=512 drive cols
                nb0 = nb * 512
                w = min(512, UPAD - nb0)
                for m in range(4):
                    psd = pp.tile([128, 512], f32, tag="ps")
                    for kk in range(2):
                        nc.tensor.matmul(
                            psd[:, :w],
                            ct_sb[kk][:, 128 * m:128 * (m + 1)],
                            ut_sb[kk][:, nb0:nb0 + w],
                            start=(kk == 0), stop=(kk == 1))
                    nc.any.tensor_copy(dt_sb[m][:, nb0:nb0 + w], psd[:, :w])

            # ---- phase A: zero-init scan over NCH chunks ----
            bmat = [cpool.tile([128, NCH], f32r, tag=f"bm{m}", name=f"bm{m}") for m in range(4)]
            st_prev = []
            for m in range(4):
                t0 = stpool.tile([128, NCH], f32r, tag=f"st{m}", name=f"st0_{m}")
                nc.vector.tensor_copy(
                    t0[:], dt_sb[m][:, 0:16 * NCH:16])
                st_prev.append(t0)
            for k in range(1, S):
                psl = [pp.tile([128, NCH], f32, tag="ps", name=f"psA{k}_{_m}") for _m in range(4)]
                for m in range(4):
                    for kk in range(4):
                        nc.tensor.matmul(
                            psl[m][:],
                            at_sb[kk][:, 128 * m:128 * (m + 1)],
                            st_prev[kk][:],
                            start=(kk == 0), stop=(kk == 3))
                st_new = []
                for m in range(4):
                    dst = (bmat[m] if k == S - 1 else
                           stpool.tile([128, NCH], f32r, tag=f"st{m}", name=f"stA{k}_{m}"))
                    nc.vector.tensor_tensor(
                        dst[:], psl[m][:],
                        dt_sb[m][:, k:k + 16 * (NCH - 1) + 1:16],
                        op=mybir.AluOpType.add)
                    st_new.append(dst)
                st_prev = st_new

            # ---- on-device powers of A^T: AT16 = (A^T)^16 by squaring ----
            uid = [0]

            def transp512(X, tag):
                uid[0] += 1
                XT = [wpool.tile([128, DZ], f32r, tag=f"{tag}{m}",
                                 name=f"{tag}{m}_{uid[0]}")
                      for m in range(4)]
                for m in range(4):
                    for kk in range(4):
                        pst = ppt.tile([128, 128], f32r, tag="pst")
                        nc.tensor.transpose(
                            pst[:], X[m][:, 128 * kk:128 * (kk + 1)],
                            id_sb[:])
                        nc.any.tensor_copy(XT[kk][:, 128 * m:128 * (m + 1)], pst[:])
                return XT

            def matmul512(XT, R, tag):
                uid[0] += 1
                Y = [wpool.tile([128, DZ], f32r, tag=f"{tag}{m}",
                                name=f"{tag}{m}_{uid[0]}")
                     for m in range(4)]
                for m in range(4):
                    ps = pp.tile([128, DZ], f32, tag="ps")
                    for kk in range(4):
                        nc.tensor.matmul(
                            ps[:],
                            XT[kk][:, 128 * m:128 * (m + 1)],
                            R[kk][:],
                            start=(kk == 0), stop=(kk == 3))
                    nc.any.tensor_copy(Y[m][:], ps[:])
                return Y

            X = at_sb
            for r in range(4):                       # AT^2, AT^4, AT^8, AT^16
                XT = transp512(X, "pwt")
                X = matmul512(XT, X, "pwx")
            at16 = [cpool.tile([128, DZ], f32r, tag=f"a16_{m}", name=f"a16_{m}")
                    for m in range(4)]
            for m in range(4):
                nc.vector.tensor_copy(at16[m][:], X[m][:])

            # ---- phase B: banded combine  w_c = sum_p (AT^16p)^T b_{c-1-p} ----
            MT = at16
            wacc = [None] * 4
            for p in range(1, K):
                if p > 1:
                    MTT = transp512(MT, "pwt")
                    MT = matmul512(MTT, at16, "pwx")
                lo = K - 1 - p
                for m in range(4):
                    ps = pp.tile([128, BCH], f32, tag="ps", name=f"psB{p}_{m}")
                    for kk in range(4):
                        nc.tensor.matmul(
                            ps[:],
                            MT[kk][:, 128 * m:128 * (m + 1)],
                            bmat[kk][:, lo:lo + BCH],
                            start=(kk == 0), stop=(kk == 3))
                    if p == 1:
                        dst = stpool.tile([128, BCH], f32, tag=f"wa{m}", name=f"wa1_{m}")
                        nc.vector.tensor_copy(dst[:], ps[:])
                    else:
                        dst = stpool.tile([128, BCH], f32, tag=f"wa{m}", name=f"wa{p}_{m}")
                        nc.vector.tensor_tensor(
                            dst[:], ps[:], wacc[m][:], op=mybir.AluOpType.add)
                    wacc[m] = dst
            w_sb = []
            for m in range(4):
                wt = cpool.tile([128, BCH], f32r, tag=f"w{m}", name=f"w{m}")
                nc.vector.tensor_tensor(
                    wt[:], wacc[m][:], bmat[m][:, K - 1:K - 1 + BCH],
                    op=mybir.AluOpType.add)
                w_sb.append(wt)

            # ---- phase C: scan 256 chunks from w_c, fused int8 output ----
            scal_sb = cpool.tile([128, 32], f32, tag="scal")
            st_prev = w_sb
            for k in range(S):
                psl = [pp.tile([128, BCH], f32, tag="ps", name=f"psC{k}_{_m}") for _m in range(4)]
                for m in range(4):
                    for kk in range(4):
                        nc.tensor.matmul(
                            psl[m][:],
                            at_sb[kk][:, 128 * m:128 * (m + 1)],
                            st_prev[kk][:],
                            start=(kk == 0), stop=(kk == 3))
                st_new = []
                st16 = []
                for m in range(4):
                    dst = stpool.tile([128, BCH], f32r, tag=f"sc{m}", name=f"stC{k}_{m}")
                    nc.vector.tensor_tensor(
                        dst[:], psl[m][:],
                        dt_sb[m][:, H + k:H + k + 16 * (BCH - 1) + 1:16],
                        op=mybir.AluOpType.add)
                    st_new.append(dst)
                    h16 = stpool.tile([128, BCH], fp16, tag=f"sh{m}", name=f"sh{k}_{m}")
                    nc.vector.tensor_copy(h16[:], dst[:].bitcast(f32))
                    st16.append(h16)
                st_prev = st_new
                # output rows t = 16*c + k (+2048h) for all 256 chunks c
                for h in range(2):
                    pso = pp.tile([128, DZ], f32, tag="ps")
                    for kk in range(4):
                        nc.tensor.matmul(
                            pso[:],
                            st16[kk][:, 128 * h:128 * (h + 1)],
                            bt_sb[kk][:],
                            start=(kk == 0), stop=(kk == 3))
                    mx = opool.tile([128, 1], f32, tag="mx")
                    nc.vector.tensor_reduce(
                        mx[:], pso[:], axis=mybir.AxisListType.X,
                        op=mybir.AluOpType.max, apply_absolute_value=True)
                    inv = opool.tile([128, 1], f32, tag="inv")
                    nc.vector.reciprocal(inv[:], mx[:])
                    nc.vector.tensor_scalar(
                        scal_sb[:, 16 * h + k:16 * h + k + 1], mx[:],
                        1.0 / 127.0, None, op0=mybir.AluOpType.mult)
                    qf = opool.tile([128, DZ], f32, tag="qf")
                    nc.vector.tensor_scalar(
                        qf[:], pso[:], inv[:], 127.0,
                        op0=mybir.AluOpType.mult, op1=mybir.AluOpType.mult)
                    qr = opool.tile([128, DZ], f32, tag="qr")
                    nc.vector.tensor_scalar(
                        qr[:], qf[:], MAGIC, MAGIC,
                        op0=mybir.AluOpType.add, op1=mybir.AluOpType.subtract)
                    ob = opool.tile([128, DZ], i8, tag="ob")
                    nc.vector.tensor_copy(ob[:], qr[:])
                    r0 = 2048 * h + k
                    nc.sync.dma_start(outq_d[r0:r0 + 2033:16, :], ob[:])
            for h in range(2):
                nc.sync.dma_start(
                    osc_d[2048 * h:2048 * (h + 1)].rearrange("(c k) -> c k", c=128),
                    scal_sb[:, 16 * h:16 * (h + 1)])
    nc.compile()
    return nc


def _build():
    if "nc" not in _CACHE:
        nc = bacc.Bacc("TRN2", target_bir_lowering=False, debug=False)
        _CACHE["nc"] = _emit(nc)
    return _CACHE["nc"]


def _host_prep(inputs_np, A, B, C):
    pad = np.zeros((H, DU), np.float32)
    up = np.concatenate([pad, inputs_np], axis=0)       # (T + H, DU)
    ct = np.ascontiguousarray(C.T).reshape(2, 128, DZ)
    bt = np.ascontiguousarray(B.T).reshape(4, 128, DZ)
    shared = {
        "at": np.ascontiguousarray(A.T),
        "cb": np.concatenate([ct, bt], axis=0).astype(np.float16),
        "id": np.eye(128, dtype=np.float32),
    }
    in_maps = []
    for i in range(NCORE):
        ut = np.zeros((DU, UPAD), np.float16)
        ut[:, :ULEN] = up[i * TLOC:i * TLOC + ULEN].T.astype(np.float16)
        in_maps.append({**shared, "ut": ut})
    return in_maps


def kernel(data, inputs, mean, A, B, C, recognition_matrix, steps=None, **kw):
    data = np.asarray(data, np.float32)
    inputs_np = np.asarray(inputs, np.float32)
    mean = np.asarray(mean, np.float32)
    A = np.asarray(A, np.float32)
    B = np.asarray(B, np.float32)
    C = np.asarray(C, np.float32)
    R = np.asarray(recognition_matrix, np.float32)

    nc = _build()
    in_maps = _host_prep(inputs_np, A, B, C)

    # host correction: output row n-1 += (A^n z0) @ B.T for n = 1..H
    A64, B64 = A.astype(np.float64), B.astype(np.float64)
    z0 = (R.astype(np.float64) @ (data[0] - mean[0]).astype(np.float64))
    zc = z0
    corr = np.empty((H, DZ), np.float64)
    for n in range(1, H + 1):
        zc = A64 @ zc
        corr[n - 1] = B64 @ zc
    corr32 = corr.astype(np.float32)

    # reference for the first 256 rows (cheap, float64) -- used as a
    # device-sanity check; a corrupted first-run execution gets retried.
    NCHK = 256
    d64 = inputs_np[:NCHK].astype(np.float64) @ C.astype(np.float64).T
    zt = z0
    ref = np.empty((NCHK, DZ), np.float64)
    for t_ in range(NCHK):
        zt = A64 @ zt + d64[t_]
        ref[t_] = B64 @ zt + mean[0].astype(np.float64)
    refn = np.linalg.norm(ref)

    for attempt in range(3):
        res = run_bass_kernel_spmd(nc, in_maps, list(range(NCORE)))
        parts = []
        for i in range(NCORE):
            q = res.results[i]["outq"].astype(np.float32)
            s = res.results[i]["osc"].reshape(TLOC, 1)
            parts.append(q * s)
        out = np.concatenate(parts, axis=0) + mean
        out[:H] += corr32
        err = np.linalg.norm(out[:NCHK].astype(np.float64) - ref) / refn
        if err < 0.05:
            break
    return out


# revision 21
# speedup vs baseline: 1.1146x; 1.1146x over previous
"""Trainium2 Bass kernel for the KalmanFilter linear recurrence.

  x = data - mean;  z0 = R @ x[0];  drive = inputs @ C.T
  z_{t+1} = A z_t + drive[t]   (T = 32768 steps, dim 512)
  result  = Z[1:] @ B.T + mean

Strategy (8 NeuronCores, sequence-parallel, no collectives):
  - ||A^k|| decays like 0.9^k (spectral radius 0.9), so the recurrence
    forgets its state after H=128 steps to ~1e-5 relative.
  - Each core owns 4096 contiguous steps, split into 256 chunks of S=16
    steps + K=8 extra "halo" chunks covering the preceding H=128 steps.
  - Phase A: batched zero-init scan over all 268 chunks (state tiles
    [512, 268], 15 matmul steps) -> per-chunk accumulated drives b_c.
  - Phase B: chunk-start states w_c = sum_{p=0}^{K-1} (A^16)^p b_{c-1-p}
    (banded combine truncated at ||A^128|| ~ 1e-6 of a unit). The
    (A^16)^p factors are computed ON DEVICE by repeated squaring (f32r),
    so no big power-matrix upload is needed.
  - Phase C: re-scan the 256 real chunks from inits w_c; each step also
    applies the output projection B.T (fp16) and quantizes rows to int8
    with a per-row (per-timestep) scale = rowmax/127 computed on device.
  - z0 only affects output rows 0..H-1 (through A^n z0); that correction
    is added on the host, so the device never sees `data`/`R`.
  Wall time over the axon tunnel is transfer-bound (~40 MB/s), so all
  I/O is minimized: u is shipped transposed in fp16, B/C in fp16, A in
  f32, output as int8 + f32 row scales; mean is added on the host.
"""
import numpy as np
import concourse.bacc as bacc
import concourse.mybir as mybir
from concourse import tile
from concourse.bass_utils import run_bass_kernel_spmd

T = 32768
DZ = 512
DU = 256
NCORE = 8
TLOC = T // NCORE          # 4096
S = 16                     # steps per chunk
BCH = TLOC // S            # 256 chunks per core
H = 128                    # halo steps (forgetting horizon)
K = H // S                 # 8 banded taps (incl. identity)
NCH = BCH + K              # 268 chunks in phase A
ULEN = TLOC + H            # 4288 drive rows per core
UPAD = ULEN                # no padding needed (u ships pre-transposed)
MAGIC = 12582912.0         # 1.5 * 2^23: float32 round-to-nearest-int trick

f32 = mybir.dt.float32
f32r = mybir.dt.float32r
fp16 = mybir.dt.float16
i8 = mybir.dt.int8

_CACHE = {}


def _emit(nc):
    ut_d = nc.dram_tensor("ut", (DU, UPAD), i8, kind="ExternalInput")
    cb_d = nc.dram_tensor("cb", (6, 128, DZ), fp16, kind="ExternalInput")
    at_d = nc.dram_tensor("at", (DZ, DZ), fp16, kind="ExternalInput")
    id_d = nc.dram_tensor("id", (128, 128), f32r, kind="ExternalInput")
    outq_d = nc.dram_tensor("outq", (TLOC, DZ), i8, kind="ExternalOutput")
    osc_d = nc.dram_tensor("osc", (TLOC,), f32, kind="ExternalOutput")

    with tile.TileContext(nc) as tc:
        with tc.tile_pool(name="const", bufs=1) as cpool, \
             tc.tile_pool(name="dt", bufs=1) as dpool, \
             tc.tile_pool(name="ut", bufs=1) as utpool, \
             tc.tile_pool(name="pw", bufs=2) as wpool, \
             tc.tile_pool(name="st", bufs=2) as stpool, \
             tc.tile_pool(name="ob", bufs=3) as opool, \
             tc.tile_pool(name="ps", bufs=6, space="PSUM") as pp, \
             tc.tile_pool(name="pst", bufs=2, space="PSUM") as ppt:

            # ---- constant loads ----
            ath = [cpool.tile([128, DZ], fp16, tag=f"ath{k}", name=f"ath{k}") for k in range(4)]
            at_sb = [cpool.tile([128, DZ], f32r, tag=f"at{k}", name=f"at{k}") for k in range(4)]
            ct_sb = [cpool.tile([128, DZ], fp16, tag=f"ct{k}", name=f"ct{k}") for k in range(2)]
            bt_sb = [cpool.tile([128, DZ], fp16, tag=f"bt{k}", name=f"bt{k}") for k in range(4)]
            id_sb = cpool.tile([128, 128], f32r, tag="id")
            for k in range(4):
                nc.sync.dma_start(ath[k][:], at_d[128 * k:128 * (k + 1), :])
                nc.sync.dma_start(bt_sb[k][:], cb_d[2 + k])
            for k in range(2):
                nc.sync.dma_start(ct_sb[k][:], cb_d[k])
            nc.sync.dma_start(id_sb[:], id_d[:])
            for k in range(4):
                nc.vector.tensor_copy(at_sb[k][:], ath[k][:])

            ut8 = [utpool.tile([128, UPAD], i8, tag=f"u8{k}", name=f"u8{k}") for k in range(2)]
            ut_sb = [utpool.tile([128, UPAD], fp16, tag=f"ut{k}", name=f"ut{k}") for k in range(2)]
            for k in range(2):
                nc.sync.dma_start(ut8[k][:], ut_d[128 * k:128 * (k + 1), :])
                nc.vector.tensor_copy(ut_sb[k][:], ut8[k][:])

            # drive rows (transposed): dT[m] holds drive.T[128m:128(m+1), :]
            dt_sb = [dpool.tile([128, UPAD], f32, tag=f"dt{m}", name=f"dt{m}") for m in range(4)]
            for nb in range((UPAD + 511) // 512):   # blocks of <=512 drive cols
                nb0 = nb * 512
                w = min(512, UPAD - nb0)
                for m in range(4):
                    psd = pp.tile([128, 512], f32, tag="ps")
                    for kk in range(2):
                        nc.tensor.matmul(
                            psd[:, :w],
                            ct_sb[kk][:, 128 * m:128 * (m + 1)],
                            ut_sb[kk][:, nb0:nb0 + w],
                            start=(kk == 0), stop=(kk == 1))
                    nc.any.tensor_copy(dt_sb[m][:, nb0:nb0 + w], psd[:, :w])

            # ---- phase A: zero-init scan over NCH chunks ----
            bmat = [cpool.tile([128, NCH], f32r, tag=f"bm{m}", name=f"bm{m}") for m in range(4)]
            st_prev = []
            for m in range(4):
                t0 = stpool.tile([128, NCH], f32r, tag=f"st{m}", name=f"st0_{m}")
                nc.vector.tensor_copy(
                    t0[:], dt_sb[m][:, 0:16 * NCH:16])
                st_prev.append(t0)
            for k in range(1, S):
                psl = [pp.tile([128, NCH], f32, tag="ps", name=f"psA{k}_{_m}") for _m in range(4)]
                for m in range(4):
                    for kk in range(4):
                        nc.tensor.matmul(
                            psl[m][:],
                            at_sb[kk][:, 128 * m:128 * (m + 1)],
                            st_prev[kk][:],
                            start=(kk == 0), stop=(kk == 3))
                st_new = []
                for m in range(4):
                    dst = (bmat[m] if k == S - 1 else
                           stpool.tile([128, NCH], f32r, tag=f"st{m}", name=f"stA{k}_{m}"))
                    nc.vector.tensor_tensor(
                        dst[:], psl[m][:],
                        dt_sb[m][:, k:k + 16 * (NCH - 1) + 1:16],
                        op=mybir.AluOpType.add)
                    st_new.append(dst)
                st_prev = st_new

            # ---- on-device powers of A^T: AT16 = (A^T)^16 by squaring ----
            uid = [0]

            def transp512(X, tag):
                uid[0] += 1
                XT = [wpool.tile([128, DZ], f32r, tag=f"{tag}{m}",
                                 name=f"{tag}{m}_{uid[0]}")
                      for m in range(4)]
                for m in range(4):
                    for kk in range(4):
                        pst = ppt.tile([128, 128], f32r, tag="pst")
                        nc.tensor.transpose(
                            pst[:], X[m][:, 128 * kk:128 * (kk + 1)],
                            id_sb[:])
                        nc.any.tensor_copy(XT[kk][:, 128 * m:128 * (m + 1)], pst[:])
                return XT

            def matmul512(XT, R, tag):
                uid[0] += 1
                Y = [wpool.tile([128, DZ], f32r, tag=f"{tag}{m}",
                                name=f"{tag}{m}_{uid[0]}")
                     for m in range(4)]
                for m in range(4):
                    ps = pp.tile([128, DZ], f32, tag="ps")
                    for kk in range(4):
                        nc.tensor.matmul(
                            ps[:],
                            XT[kk][:, 128 * m:128 * (m + 1)],
                            R[kk][:],
                            start=(kk == 0), stop=(kk == 3))
                    nc.any.tensor_copy(Y[m][:], ps[:])
                return Y

            X = at_sb
            for r in range(4):                       # AT^2, AT^4, AT^8, AT^16
                XT = transp512(X, "pwt")
                X = matmul512(XT, X, "pwx")
            at16 = [cpool.tile([128, DZ], f32r, tag=f"a16_{m}", name=f"a16_{m}")
                    for m in range(4)]
            for m in range(4):
                nc.vector.tensor_copy(at16[m][:], X[m][:])

            # ---- phase B: banded combine  w_c = sum_p (AT^16p)^T b_{c-1-p} ----
            MT = at16
            wacc = [None] * 4
            for p in range(1, K):
                if p > 1:
                    MTT = transp512(MT, "pwt")
                    MT = matmul512(MTT, at16, "pwx")
                lo = K - 1 - p
                for m in range(4):
                    ps = pp.tile([128, BCH], f32, tag="ps", name=f"psB{p}_{m}")
                    for kk in range(4):
                        nc.tensor.matmul(
                            ps[:],
                            MT[kk][:, 128 * m:128 * (m + 1)],
                            bmat[kk][:, lo:lo + BCH],
                            start=(kk == 0), stop=(kk == 3))
                    if p == 1:
                        dst = stpool.tile([128, BCH], f32, tag=f"wa{m}", name=f"wa1_{m}")
                        nc.vector.tensor_copy(dst[:], ps[:])
                    else:
                        dst = stpool.tile([128, BCH], f32, tag=f"wa{m}", name=f"wa{p}_{m}")
                        nc.vector.tensor_tensor(
                            dst[:], ps[:], wacc[m][:], op=mybir.AluOpType.add)
                    wacc[m] = dst
            w_sb = []
            for m in range(4):
                wt = cpool.tile([128, BCH], f32r, tag=f"w{m}", name=f"w{m}")
                nc.vector.tensor_tensor(
                    wt[:], wacc[m][:], bmat[m][:, K - 1:K - 1 + BCH],
                    op=mybir.AluOpType.add)
                w_sb.append(wt)

            # ---- phase C: scan 256 chunks from w_c, fused int8 output ----
            scal_sb = cpool.tile([128, 32], f32, tag="scal")
            st_prev = w_sb
            for k in range(S):
                psl = [pp.tile([128, BCH], f32, tag="ps", name=f"psC{k}_{_m}") for _m in range(4)]
                for m in range(4):
                    for kk in range(4):
                        nc.tensor.matmul(
                            psl[m][:],
                            at_sb[kk][:, 128 * m:128 * (m + 1)],
                            st_prev[kk][:],
                            start=(kk == 0), stop=(kk == 3))
                st_new = []
                st16 = []
                for m in range(4):
                    dst = stpool.tile([128, BCH], f32r, tag=f"sc{m}", name=f"stC{k}_{m}")
                    nc.vector.tensor_tensor(
                        dst[:], psl[m][:],
                        dt_sb[m][:, H + k:H + k + 16 * (BCH - 1) + 1:16],
                        op=mybir.AluOpType.add)
                    st_new.append(dst)
                    h16 = stpool.tile([128, BCH], fp16, tag=f"sh{m}", name=f"sh{k}_{m}")
                    nc.vector.tensor_copy(h16[:], dst[:].bitcast(f32))
                    st16.append(h16)
                st_prev = st_new
                # output rows t = 16*c + k (+2048h) for all 256 chunks c
                for h in range(2):
                    pso = pp.tile([128, DZ], f32, tag="ps")
                    for kk in range(4):
                        nc.tensor.matmul(
                            pso[:],
                            st16[kk][:, 128 * h:128 * (h + 1)],
                            bt_sb[kk][:],
                            start=(kk == 0), stop=(kk == 3))
                    mx = opool.tile([128, 1], f32, tag="mx")
                    nc.vector.tensor_reduce(
                        mx[:], pso[:], axis=mybir.AxisListType.X,
                        op=mybir.AluOpType.max, apply_absolute_value=True)
                    inv = opool.tile([128, 1], f32, tag="inv")
                    nc.vector.reciprocal(inv[:], mx[:])
                    nc.vector.tensor_scalar(
                        scal_sb[:, 16 * h + k:16 * h + k + 1], mx[:],
                        1.0 / 127.0, None, op0=mybir.AluOpType.mult)
                    qf = opool.tile([128, DZ], f32, tag="qf")
                    nc.vector.tensor_scalar(
                        qf[:], pso[:], inv[:], 127.0,
                        op0=mybir.AluOpType.mult, op1=mybir.AluOpType.mult)
                    qr = opool.tile([128, DZ], f32, tag="qr")
                    nc.vector.tensor_scalar(
                        qr[:], qf[:], MAGIC, MAGIC,
                        op0=mybir.AluOpType.add, op1=mybir.AluOpType.subtract)
                    ob = opool.tile([128, DZ], i8, tag="ob")
                    nc.vector.tensor_copy(ob[:], qr[:])
                    r0 = 2048 * h + k
                    nc.sync.dma_start(outq_d[r0:r0 + 2033:16, :], ob[:])
            for h in range(2):
                nc.sync.dma_start(
                    osc_d[2048 * h:2048 * (h + 1)].rearrange("(c k) -> c k", c=128),
                    scal_sb[:, 16 * h:16 * (h + 1)])
    nc.compile()
    return nc


def _build():
    if "nc" not in _CACHE:
        nc = bacc.Bacc("TRN2", target_bir_lowering=False, debug=False)
        _CACHE["nc"] = _emit(nc)
    return _CACHE["nc"]


def _host_prep(inputs_np, A, B, C):
    # u quantized to int8 with one scale per u-dim; the scale is folded
    # into C.T on the host so the device only does an int8->fp16 convert.
    usc = np.abs(inputs_np).max(axis=0) / 127.0         # (DU,)
    usc = np.maximum(usc, 1e-30)
    u8 = np.round(inputs_np / usc).clip(-127, 127).astype(np.int8)
    pad = np.zeros((H, DU), np.int8)
    up = np.concatenate([pad, u8], axis=0)              # (T + H, DU)
    ct = (np.ascontiguousarray(C.T) * usc[:, None]).reshape(2, 128, DZ)
    bt = np.ascontiguousarray(B.T).reshape(4, 128, DZ)
    shared = {
        "at": np.ascontiguousarray(A.T).astype(np.float16),
        "cb": np.concatenate([ct, bt], axis=0).astype(np.float16),
        "id": np.eye(128, dtype=np.float32),
    }
    in_maps = []
    for i in range(NCORE):
        ut = np.ascontiguousarray(up[i * TLOC:i * TLOC + ULEN].T)
        in_maps.append({**shared, "ut": ut})
    return in_maps


def kernel(data, inputs, mean, A, B, C, recognition_matrix, steps=None, **kw):
    data = np.asarray(data, np.float32)
    inputs_np = np.asarray(inputs, np.float32)
    mean = np.asarray(mean, np.float32)
    A = np.asarray(A, np.float32)
    B = np.asarray(B, np.float32)
    C = np.asarray(C, np.float32)
    R = np.asarray(recognition_matrix, np.float32)

    nc = _build()
    in_maps = _host_prep(inputs_np, A, B, C)

    # host correction: output row n-1 += (A^n z0) @ B.T for n = 1..H
    A64, B64 = A.astype(np.float64), B.astype(np.float64)
    z0 = (R.astype(np.float64) @ (data[0] - mean[0]).astype(np.float64))
    zc = z0
    corr = np.empty((H, DZ), np.float64)
    for n in range(1, H + 1):
        zc = A64 @ zc
        corr[n - 1] = B64 @ zc
    corr32 = corr.astype(np.float32)

    # reference for the first 256 rows (cheap, float64) -- used as a
    # device-sanity check; a corrupted first-run execution gets retried.
    NCHK = 256
    d64 = inputs_np[:NCHK].astype(np.float64) @ C.astype(np.float64).T
    zt = z0
    ref = np.empty((NCHK, DZ), np.float64)
    for t_ in range(NCHK):
        zt = A64 @ zt + d64[t_]
        ref[t_] = B64 @ zt + mean[0].astype(np.float64)
    refn = np.linalg.norm(ref)

    for attempt in range(3):
        res = run_bass_kernel_spmd(nc, in_maps, list(range(NCORE)))
        parts = []
        for i in range(NCORE):
            q = res.results[i]["outq"].astype(np.float32)
            s = res.results[i]["osc"].reshape(TLOC, 1)
            parts.append(q * s)
        out = np.concatenate(parts, axis=0) + mean
        out[:H] += corr32
        err = np.linalg.norm(out[:NCHK].astype(np.float64) - ref) / refn
        if err < 0.05:
            break
    return out


# revision 23
# speedup vs baseline: 1.3371x; 1.1996x over previous
"""Trainium2 Bass kernel for the KalmanFilter linear recurrence.

  x = data - mean;  z0 = R @ x[0];  drive = inputs @ C.T
  z_{t+1} = A z_t + drive[t]   (T = 32768 steps, dim 512)
  result  = Z[1:] @ B.T + mean

Strategy (8 NeuronCores, sequence-parallel, no collectives):
  - ||A^k|| decays like 0.9^k (spectral radius 0.9), so the recurrence
    forgets its state after H=128 steps to ~1e-5 relative.
  - Each core owns 4096 contiguous steps, split into 256 chunks of S=16
    steps + K=8 extra "halo" chunks covering the preceding H=128 steps.
  - Phase A: batched zero-init scan over all 268 chunks (state tiles
    [512, 268], 15 matmul steps) -> per-chunk accumulated drives b_c.
  - Phase B: chunk-start states w_c = sum_{p=0}^{K-1} (A^16)^p b_{c-1-p}
    (banded combine truncated at ||A^128|| ~ 1e-6 of a unit). The
    (A^16)^p factors are computed ON DEVICE by repeated squaring (f32r),
    so no big power-matrix upload is needed.
  - Phase C: re-scan the 256 real chunks from inits w_c; each step also
    applies the output projection B.T (fp16) and quantizes rows to int8
    with a per-row (per-timestep) scale = rowmax/127 computed on device.
  - z0 only affects output rows 0..H-1 (through A^n z0); that correction
    is added on the host, so the device never sees `data`/`R`.
  Wall time over the axon tunnel is transfer-bound (~40 MB/s), so all
  I/O is minimized: u is shipped transposed in fp16, B/C in fp16, A in
  f32, output as int8 + f32 row scales; mean is added on the host.
"""
import numpy as np
import concourse.bacc as bacc
import concourse.mybir as mybir
from concourse import tile
from concourse.bass_utils import run_bass_kernel_spmd

T = 32768
DZ = 512
DU = 256
NCORE = 8
TLOC = T // NCORE          # 4096
S = 16                     # steps per chunk
BCH = TLOC // S            # 256 chunks per core
H = 128                    # halo steps (forgetting horizon)
K = H // S                 # 8 banded taps (incl. identity)
NCH = BCH + K              # 268 chunks in phase A
ULEN = TLOC + H            # 4288 drive rows per core
UPAD = ULEN                # no padding needed (u ships pre-transposed)
MAGIC = 12582912.0         # 1.5 * 2^23: float32 round-to-nearest-int trick

f32 = mybir.dt.float32
f32r = mybir.dt.float32r
fp16 = mybir.dt.float16
i8 = mybir.dt.int8

_CACHE = {}


def _emit(nc):
    ut_d = nc.dram_tensor("ut", (DU, UPAD), i8, kind="ExternalInput")
    cs_d = nc.dram_tensor("cs", (1408 // NCORE, DZ), fp16, kind="ExternalInput")
    outq_d = nc.dram_tensor("outq", (TLOC, DZ), i8, kind="ExternalOutput")
    osc_d = nc.dram_tensor("osc", (TLOC,), f32, kind="ExternalOutput")

    with tile.TileContext(nc) as tc:
        with tc.tile_pool(name="const", bufs=1) as cpool, \
             tc.tile_pool(name="dram", bufs=1, space="DRAM") as drpool, \
             tc.tile_pool(name="dt", bufs=1) as dpool, \
             tc.tile_pool(name="ut", bufs=1) as utpool, \
             tc.tile_pool(name="pw", bufs=2) as wpool, \
             tc.tile_pool(name="st", bufs=2) as stpool, \
             tc.tile_pool(name="ob", bufs=3) as opool, \
             tc.tile_pool(name="ps", bufs=6, space="PSUM") as pp, \
             tc.tile_pool(name="pst", bufs=2, space="PSUM") as ppt:

            # ---- const blob: shard upload + on-device AllGather ----
            # blob rows: [0:512] A.T | [512:768] C.T*usc | [768:1280] B.T
            #            | [1280:1408] identity (cols 0:128)
            ib = drpool.tile([1408 // NCORE, DZ], fp16)
            gb = drpool.tile([1408, DZ], fp16)
            nc.gpsimd.dma_start(ib[:], cs_d[:])
            nc.gpsimd.collective_compute(
                "AllGather", mybir.AluOpType.bypass,
                replica_groups=[list(range(NCORE))],
                ins=[ib.opt()], outs=[gb.opt()])

            ath = [cpool.tile([128, DZ], fp16, tag=f"ath{k}", name=f"ath{k}") for k in range(4)]
            at_sb = [cpool.tile([128, DZ], f32r, tag=f"at{k}", name=f"at{k}") for k in range(4)]
            ct_sb = [cpool.tile([128, DZ], fp16, tag=f"ct{k}", name=f"ct{k}") for k in range(2)]
            bt_sb = [cpool.tile([128, DZ], fp16, tag=f"bt{k}", name=f"bt{k}") for k in range(4)]
            idh = cpool.tile([128, 128], fp16, tag="idh")
            id_sb = cpool.tile([128, 128], f32r, tag="id")
            for k in range(4):
                nc.sync.dma_start(ath[k][:], gb[128 * k:128 * (k + 1), :])
                nc.sync.dma_start(bt_sb[k][:], gb[768 + 128 * k:768 + 128 * (k + 1), :])
            for k in range(2):
                nc.sync.dma_start(ct_sb[k][:], gb[512 + 128 * k:512 + 128 * (k + 1), :])
            nc.sync.dma_start(idh[:], gb[1280:1408, 0:128])
            nc.vector.tensor_copy(id_sb[:], idh[:])
            for k in range(4):
                nc.vector.tensor_copy(at_sb[k][:], ath[k][:])

            ut8 = [utpool.tile([128, UPAD], i8, tag=f"u8{k}", name=f"u8{k}") for k in range(2)]
            ut_sb = [utpool.tile([128, UPAD], fp16, tag=f"ut{k}", name=f"ut{k}") for k in range(2)]
            for k in range(2):
                nc.sync.dma_start(ut8[k][:], ut_d[128 * k:128 * (k + 1), :])
                nc.vector.tensor_copy(ut_sb[k][:], ut8[k][:])

            # drive rows (transposed): dT[m] holds drive.T[128m:128(m+1), :]
            dt_sb = [dpool.tile([128, UPAD], f32, tag=f"dt{m}", name=f"dt{m}") for m in range(4)]
            for nb in range((UPAD + 511) // 512):   # blocks of <=512 drive cols
                nb0 = nb * 512
                w = min(512, UPAD - nb0)
                for m in range(4):
                    psd = pp.tile([128, 512], f32, tag="ps")
                    for kk in range(2):
                        nc.tensor.matmul(
                            psd[:, :w],
                            ct_sb[kk][:, 128 * m:128 * (m + 1)],
                            ut_sb[kk][:, nb0:nb0 + w],
                            start=(kk == 0), stop=(kk == 1))
                    nc.any.tensor_copy(dt_sb[m][:, nb0:nb0 + w], psd[:, :w])

            # ---- phase A: zero-init scan over NCH chunks ----
            bmat = [cpool.tile([128, NCH], f32r, tag=f"bm{m}", name=f"bm{m}") for m in range(4)]
            st_prev = []
            for m in range(4):
                t0 = stpool.tile([128, NCH], f32r, tag=f"st{m}", name=f"st0_{m}")
                nc.vector.tensor_copy(
                    t0[:], dt_sb[m][:, 0:16 * NCH:16])
                st_prev.append(t0)
            for k in range(1, S):
                psl = [pp.tile([128, NCH], f32, tag="ps", name=f"psA{k}_{_m}") for _m in range(4)]
                for m in range(4):
                    for kk in range(4):
                        nc.tensor.matmul(
                            psl[m][:],
                            at_sb[kk][:, 128 * m:128 * (m + 1)],
                            st_prev[kk][:],
                            start=(kk == 0), stop=(kk == 3))
                st_new = []
                for m in range(4):
                    dst = (bmat[m] if k == S - 1 else
                           stpool.tile([128, NCH], f32r, tag=f"st{m}", name=f"stA{k}_{m}"))
                    nc.vector.tensor_tensor(
                        dst[:], psl[m][:],
                        dt_sb[m][:, k:k + 16 * (NCH - 1) + 1:16],
                        op=mybir.AluOpType.add)
                    st_new.append(dst)
                st_prev = st_new

            # ---- on-device powers of A^T: AT16 = (A^T)^16 by squaring ----
            uid = [0]

            def transp512(X, tag):
                uid[0] += 1
                XT = [wpool.tile([128, DZ], f32r, tag=f"{tag}{m}",
                                 name=f"{tag}{m}_{uid[0]}")
                      for m in range(4)]
                for m in range(4):
                    for kk in range(4):
                        pst = ppt.tile([128, 128], f32r, tag="pst")
                        nc.tensor.transpose(
                            pst[:], X[m][:, 128 * kk:128 * (kk + 1)],
                            id_sb[:])
                        nc.any.tensor_copy(XT[kk][:, 128 * m:128 * (m + 1)], pst[:])
                return XT

            def matmul512(XT, R, tag):
                uid[0] += 1
                Y = [wpool.tile([128, DZ], f32r, tag=f"{tag}{m}",
                                name=f"{tag}{m}_{uid[0]}")
                     for m in range(4)]
                for m in range(4):
                    ps = pp.tile([128, DZ], f32, tag="ps")
                    for kk in range(4):
                        nc.tensor.matmul(
                            ps[:],
                            XT[kk][:, 128 * m:128 * (m + 1)],
                            R[kk][:],
                            start=(kk == 0), stop=(kk == 3))
                    nc.any.tensor_copy(Y[m][:], ps[:])
                return Y

            X = at_sb
            for r in range(4):                       # AT^2, AT^4, AT^8, AT^16
                XT = transp512(X, "pwt")
                X = matmul512(XT, X, "pwx")
            at16 = [cpool.tile([128, DZ], f32r, tag=f"a16_{m}", name=f"a16_{m}")
                    for m in range(4)]
            for m in range(4):
                nc.vector.tensor_copy(at16[m][:], X[m][:])

            # ---- phase B: banded combine  w_c = sum_p (AT^16p)^T b_{c-1-p} ----
            MT = at16
            wacc = [None] * 4
            for p in range(1, K):
                if p > 1:
                    MTT = transp512(MT, "pwt")
                    MT = matmul512(MTT, at16, "pwx")
                lo = K - 1 - p
                for m in range(4):
                    ps = pp.tile([128, BCH], f32, tag="ps", name=f"psB{p}_{m}")
                    for kk in range(4):
                        nc.tensor.matmul(
                            ps[:],
                            MT[kk][:, 128 * m:128 * (m + 1)],
                            bmat[kk][:, lo:lo + BCH],
                            start=(kk == 0), stop=(kk == 3))
                    if p == 1:
                        dst = stpool.tile([128, BCH], f32, tag=f"wa{m}", name=f"wa1_{m}")
                        nc.vector.tensor_copy(dst[:], ps[:])
                    else:
                        dst = stpool.tile([128, BCH], f32, tag=f"wa{m}", name=f"wa{p}_{m}")
                        nc.vector.tensor_tensor(
                            dst[:], ps[:], wacc[m][:], op=mybir.AluOpType.add)
                    wacc[m] = dst
            w_sb = []
            for m in range(4):
                wt = cpool.tile([128, BCH], f32r, tag=f"w{m}", name=f"w{m}")
                nc.vector.tensor_tensor(
                    wt[:], wacc[m][:], bmat[m][:, K - 1:K - 1 + BCH],
                    op=mybir.AluOpType.add)
                w_sb.append(wt)

            # ---- phase C: scan 256 chunks from w_c, fused int8 output ----
            scal_sb = cpool.tile([128, 32], f32, tag="scal")
            st_prev = w_sb
            for k in range(S):
                psl = [pp.tile([128, BCH], f32, tag="ps", name=f"psC{k}_{_m}") for _m in range(4)]
                for m in range(4):
                    for kk in range(4):
                        nc.tensor.matmul(
                            psl[m][:],
                            at_sb[kk][:, 128 * m:128 * (m + 1)],
                            st_prev[kk][:],
                            start=(kk == 0), stop=(kk == 3))
                st_new = []
                st16 = []
                for m in range(4):
                    dst = stpool.tile([128, BCH], f32r, tag=f"sc{m}", name=f"stC{k}_{m}")
                    nc.vector.tensor_tensor(
                        dst[:], psl[m][:],
                        dt_sb[m][:, H + k:H + k + 16 * (BCH - 1) + 1:16],
                        op=mybir.AluOpType.add)
                    st_new.append(dst)
                    h16 = stpool.tile([128, BCH], fp16, tag=f"sh{m}", name=f"sh{k}_{m}")
                    nc.vector.tensor_copy(h16[:], dst[:].bitcast(f32))
                    st16.append(h16)
                st_prev = st_new
                # output rows t = 16*c + k (+2048h) for all 256 chunks c
                for h in range(2):
                    pso = pp.tile([128, DZ], f32, tag="ps")
                    for kk in range(4):
                        nc.tensor.matmul(
                            pso[:],
                            st16[kk][:, 128 * h:128 * (h + 1)],
                            bt_sb[kk][:],
                            start=(kk == 0), stop=(kk == 3))
                    mx = opool.tile([128, 1], f32, tag="mx")
                    nc.vector.tensor_reduce(
                        mx[:], pso[:], axis=mybir.AxisListType.X,
                        op=mybir.AluOpType.max, apply_absolute_value=True)
                    inv = opool.tile([128, 1], f32, tag="inv")
                    nc.vector.reciprocal(inv[:], mx[:])
                    nc.vector.tensor_scalar(
                        scal_sb[:, 16 * h + k:16 * h + k + 1], mx[:],
                        1.0 / 127.0, None, op0=mybir.AluOpType.mult)
                    qf = opool.tile([128, DZ], f32, tag="qf")
                    nc.vector.tensor_scalar(
                        qf[:], pso[:], inv[:], 127.0,
                        op0=mybir.AluOpType.mult, op1=mybir.AluOpType.mult)
                    qr = opool.tile([128, DZ], f32, tag="qr")
                    nc.vector.tensor_scalar(
                        qr[:], qf[:], MAGIC, MAGIC,
                        op0=mybir.AluOpType.add, op1=mybir.AluOpType.subtract)
                    ob = opool.tile([128, DZ], i8, tag="ob")
                    nc.vector.tensor_copy(ob[:], qr[:])
                    r0 = 2048 * h + k
                    nc.sync.dma_start(outq_d[r0:r0 + 2033:16, :], ob[:])
            for h in range(2):
                nc.sync.dma_start(
                    osc_d[2048 * h:2048 * (h + 1)].rearrange("(c k) -> c k", c=128),
                    scal_sb[:, 16 * h:16 * (h + 1)])
    nc.compile()
    return nc


def _build():
    if "nc" not in _CACHE:
        nc = bacc.Bacc("TRN2", target_bir_lowering=False, debug=False)
        _CACHE["nc"] = _emit(nc)
    return _CACHE["nc"]


def _host_prep(inputs_np, A, B, C):
    # u quantized to int8 with one scale per u-dim; the scale is folded
    # into C.T on the host so the device only does an int8->fp16 convert.
    usc = np.abs(inputs_np).max(axis=0) / 127.0         # (DU,)
    usc = np.maximum(usc, 1e-30)
    u8 = np.round(inputs_np / usc).clip(-127, 127).astype(np.int8)
    pad = np.zeros((H, DU), np.int8)
    up = np.concatenate([pad, u8], axis=0)              # (T + H, DU)
    blob = np.zeros((1408, DZ), np.float16)
    blob[0:512] = A.T.astype(np.float16)
    blob[512:768] = (C.T * usc[:, None]).astype(np.float16)
    blob[768:1280] = B.T.astype(np.float16)
    blob[1280:1408, 0:128] = np.eye(128, dtype=np.float16)
    shard = 1408 // NCORE
    in_maps = []
    for i in range(NCORE):
        ut = np.ascontiguousarray(up[i * TLOC:i * TLOC + ULEN].T)
        in_maps.append({"ut": ut, "cs": blob[shard * i:shard * (i + 1)]})
    return in_maps


def kernel(data, inputs, mean, A, B, C, recognition_matrix, steps=None, **kw):
    data = np.asarray(data, np.float32)
    inputs_np = np.asarray(inputs, np.float32)
    mean = np.asarray(mean, np.float32)
    A = np.asarray(A, np.float32)
    B = np.asarray(B, np.float32)
    C = np.asarray(C, np.float32)
    R = np.asarray(recognition_matrix, np.float32)

    nc = _build()
    in_maps = _host_prep(inputs_np, A, B, C)

    # host correction: output row n-1 += (A^n z0) @ B.T for n = 1..H
    A64, B64 = A.astype(np.float64), B.astype(np.float64)
    z0 = (R.astype(np.float64) @ (data[0] - mean[0]).astype(np.float64))
    zc = z0
    corr = np.empty((H, DZ), np.float64)
    for n in range(1, H + 1):
        zc = A64 @ zc
        corr[n - 1] = B64 @ zc
    corr32 = corr.astype(np.float32)

    # reference for the first 256 rows (cheap, float64) -- used as a
    # device-sanity check; a corrupted first-run execution gets retried.
    NCHK = 256
    d64 = inputs_np[:NCHK].astype(np.float64) @ C.astype(np.float64).T
    zt = z0
    ref = np.empty((NCHK, DZ), np.float64)
    for t_ in range(NCHK):
        zt = A64 @ zt + d64[t_]
        ref[t_] = B64 @ zt + mean[0].astype(np.float64)
    refn = np.linalg.norm(ref)

    for attempt in range(3):
        res = run_bass_kernel_spmd(nc, in_maps, list(range(NCORE)))
        parts = []
        for i in range(NCORE):
            q = res.results[i]["outq"].astype(np.float32)
            s = res.results[i]["osc"].reshape(TLOC, 1)
            parts.append(q * s)
        out = np.concatenate(parts, axis=0) + mean
        out[:H] += corr32
        err = np.linalg.norm(out[:NCHK].astype(np.float64) - ref) / refn
        if err < 0.05:
            break
    return out


# revision 31
# speedup vs baseline: 1.4855x; 1.1110x over previous
"""Trainium2 Bass kernel for the KalmanFilter linear recurrence.

  x = data - mean;  z0 = R @ x[0];  drive = inputs @ C.T
  z_{t+1} = A z_t + drive[t]   (T = 32768 steps, dim 512)
  result  = Z[1:] @ B.T + mean

Strategy (8 NeuronCores, sequence-parallel, no collectives):
  - ||A^k|| decays like 0.9^k (spectral radius 0.9), so the recurrence
    forgets its state after H=128 steps to ~1e-5 relative.
  - Each core owns 4096 contiguous steps, split into 256 chunks of S=16
    steps + K=8 extra "halo" chunks covering the preceding H=128 steps.
  - Phase A: batched zero-init scan over all 268 chunks (state tiles
    [512, 268], 15 matmul steps) -> per-chunk accumulated drives b_c.
  - Phase B: chunk-start states w_c = sum_{p=0}^{K-1} (A^16)^p b_{c-1-p}
    (banded combine truncated at ||A^128|| ~ 1e-6 of a unit). The
    (A^16)^p factors are computed ON DEVICE by repeated squaring (f32r),
    so no big power-matrix upload is needed.
  - Phase C: re-scan the 256 real chunks from inits w_c; each step also
    applies the output projection B.T (fp16) and quantizes rows to int8
    with a per-row (per-timestep) scale = rowmax/127 computed on device.
  - z0 only affects output rows 0..H-1 (through A^n z0); that correction
    is added on the host, so the device never sees `data`/`R`.
  Wall time over the axon tunnel is transfer-bound (~40 MB/s), so all
  I/O is minimized: u is shipped transposed in fp16, B/C in fp16, A in
  f32, output as int8 + f32 row scales; mean is added on the host.
"""
import numpy as np
import concourse.bacc as bacc
import concourse.mybir as mybir
from concourse import tile
from concourse.bass_utils import run_bass_kernel_spmd

T = 32768
DZ = 512
DU = 256
NCORE = 8
TLOC = T // NCORE          # 4096
S = 16                     # steps per chunk
BCH = TLOC // S            # 256 chunks per core
H = 128                    # halo steps (forgetting horizon)
K = H // S                 # 8 banded taps (incl. identity)
NCH = BCH + K              # 268 chunks in phase A
ULEN = TLOC + H            # 4288 drive rows per core
UPAD = ULEN                # no padding needed (u ships pre-transposed)
MAGIC = 12582912.0         # 1.5 * 2^23: float32 round-to-nearest-int trick

f32 = mybir.dt.float32
f32r = mybir.dt.float32r
fp16 = mybir.dt.float16
i8 = mybir.dt.int8

_CACHE = {}


def _emit(nc):
    # flat int8 input: u.T int8 (DU*UPAD bytes) | const-blob shard (fp16 bytes)
    CSROWS = 1408 // NCORE
    ut_d = nc.dram_tensor("ut", (DU * UPAD + CSROWS * DZ * 2,), i8,
                          kind="ExternalInput")
    # flat int8 output: rows 0..TLOC-1 = int8 data; the last 32 rows hold
    # the TLOC per-row f32 scales (bitcast), in plain row order.
    outq_d = nc.dram_tensor("outq", ((TLOC + 32) * DZ,), i8, kind="ExternalOutput")

    with tile.TileContext(nc) as tc:
        with tc.tile_pool(name="const", bufs=1) as cpool, \
             tc.tile_pool(name="dram", bufs=1, space="DRAM") as drpool, \
             tc.tile_pool(name="dt", bufs=1) as dpool, \
             tc.tile_pool(name="ut", bufs=1) as utpool, \
             tc.tile_pool(name="pw", bufs=2) as wpool, \
             tc.tile_pool(name="st", bufs=2) as stpool, \
             tc.tile_pool(name="ob", bufs=3) as opool, \
             tc.tile_pool(name="ps", bufs=6, space="PSUM") as pp, \
             tc.tile_pool(name="pst", bufs=2, space="PSUM") as ppt:

            # ---- const blob: shard upload + on-device AllGather ----
            # blob rows: [0:512] A.T | [512:768] C.T*usc | [768:1280] B.T
            #            | [1280:1408] identity (cols 0:128)
            ib = drpool.tile([CSROWS, 2 * DZ], i8)
            gb = drpool.tile([1408, 2 * DZ], i8)
            nc.gpsimd.dma_start(
                ib[:], ut_d[DU * UPAD:].rearrange("(p q) -> p q", q=2 * DZ))
            nc.gpsimd.collective_compute(
                "AllGather", mybir.AluOpType.bypass,
                replica_groups=[list(range(NCORE))],
                ins=[ib.opt()], outs=[gb.opt()])

            ath = [cpool.tile([128, DZ], fp16, tag=f"ath{k}", name=f"ath{k}") for k in range(4)]
            at_sb = [cpool.tile([128, DZ], f32r, tag=f"at{k}", name=f"at{k}") for k in range(4)]
            ct_sb = [cpool.tile([128, DZ], fp16, tag=f"ct{k}", name=f"ct{k}") for k in range(2)]
            bt_sb = [cpool.tile([128, DZ], fp16, tag=f"bt{k}", name=f"bt{k}") for k in range(4)]
            idh = cpool.tile([128, 128], fp16, tag="idh")
            id_sb = cpool.tile([128, 128], f32r, tag="id")
            for k in range(4):
                nc.sync.dma_start(ath[k][:], gb[128 * k:128 * (k + 1), :].bitcast(fp16))
                nc.sync.dma_start(bt_sb[k][:],
                                  gb[768 + 128 * k:768 + 128 * (k + 1), :].bitcast(fp16))
            for k in range(2):
                nc.sync.dma_start(ct_sb[k][:],
                                  gb[512 + 128 * k:512 + 128 * (k + 1), :].bitcast(fp16))
            nc.sync.dma_start(idh[:], gb[1280:1408, 0:256].bitcast(fp16))
            nc.vector.tensor_copy(id_sb[:], idh[:])
            for k in range(4):
                nc.vector.tensor_copy(at_sb[k][:], ath[k][:])

            ut8 = [utpool.tile([128, UPAD], i8, tag=f"u8{k}", name=f"u8{k}") for k in range(2)]
            ut_sb = [utpool.tile([128, UPAD], fp16, tag=f"ut{k}", name=f"ut{k}") for k in range(2)]
            for k in range(2):
                nc.sync.dma_start(
                    ut8[k][:],
                    ut_d[128 * k * UPAD:128 * (k + 1) * UPAD].rearrange(
                        "(p q) -> p q", q=UPAD))
                nc.vector.tensor_copy(ut_sb[k][:], ut8[k][:])

            # drive rows (transposed): dT[m] holds drive.T[128m:128(m+1), :]
            dt_sb = [dpool.tile([128, UPAD], f32, tag=f"dt{m}", name=f"dt{m}") for m in range(4)]
            for nb in range((UPAD + 511) // 512):   # blocks of <=512 drive cols
                nb0 = nb * 512
                w = min(512, UPAD - nb0)
                for m in range(4):
                    psd = pp.tile([128, 512], f32, tag="ps")
                    for kk in range(2):
                        nc.tensor.matmul(
                            psd[:, :w],
                            ct_sb[kk][:, 128 * m:128 * (m + 1)],
                            ut_sb[kk][:, nb0:nb0 + w],
                            start=(kk == 0), stop=(kk == 1))
                    nc.any.tensor_copy(dt_sb[m][:, nb0:nb0 + w], psd[:, :w])

            # ---- phase A: zero-init scan over NCH chunks ----
            bmat = [cpool.tile([128, NCH], f32r, tag=f"bm{m}", name=f"bm{m}") for m in range(4)]
            st_prev = []
            for m in range(4):
                t0 = stpool.tile([128, NCH], f32r, tag=f"st{m}", name=f"st0_{m}")
                nc.vector.tensor_copy(
                    t0[:], dt_sb[m][:, 0:16 * NCH:16])
                st_prev.append(t0)
            for k in range(1, S):
                psl = [pp.tile([128, NCH], f32, tag="ps", name=f"psA{k}_{_m}") for _m in range(4)]
                for m in range(4):
                    for kk in range(4):
                        nc.tensor.matmul(
                            psl[m][:],
                            at_sb[kk][:, 128 * m:128 * (m + 1)],
                            st_prev[kk][:],
                            start=(kk == 0), stop=(kk == 3))
                st_new = []
                for m in range(4):
                    dst = (bmat[m] if k == S - 1 else
                           stpool.tile([128, NCH], f32r, tag=f"st{m}", name=f"stA{k}_{m}"))
                    nc.vector.tensor_tensor(
                        dst[:], psl[m][:],
                        dt_sb[m][:, k:k + 16 * (NCH - 1) + 1:16],
                        op=mybir.AluOpType.add)
                    st_new.append(dst)
                st_prev = st_new

            # ---- on-device powers of A^T: AT16 = (A^T)^16 by squaring ----
            uid = [0]

            def transp512(X, tag):
                uid[0] += 1
                XT = [wpool.tile([128, DZ], f32r, tag=f"{tag}{m}",
                                 name=f"{tag}{m}_{uid[0]}")
                      for m in range(4)]
                for m in range(4):
                    for kk in range(4):
                        pst = ppt.tile([128, 128], f32r, tag="pst")
                        nc.tensor.transpose(
                            pst[:], X[m][:, 128 * kk:128 * (kk + 1)],
                            id_sb[:])
                        nc.any.tensor_copy(XT[kk][:, 128 * m:128 * (m + 1)], pst[:])
                return XT

            def matmul512(XT, R, tag):
                uid[0] += 1
                Y = [wpool.tile([128, DZ], f32r, tag=f"{tag}{m}",
                                name=f"{tag}{m}_{uid[0]}")
                     for m in range(4)]
                for m in range(4):
                    ps = pp.tile([128, DZ], f32, tag="ps")
                    for kk in range(4):
                        nc.tensor.matmul(
                            ps[:],
                            XT[kk][:, 128 * m:128 * (m + 1)],
                            R[kk][:],
                            start=(kk == 0), stop=(kk == 3))
                    nc.any.tensor_copy(Y[m][:], ps[:])
                return Y

            X = at_sb
            for r in range(4):                       # AT^2, AT^4, AT^8, AT^16
                XT = transp512(X, "pwt")
                X = matmul512(XT, X, "pwx")
            at16 = [cpool.tile([128, DZ], f32r, tag=f"a16_{m}", name=f"a16_{m}")
                    for m in range(4)]
            for m in range(4):
                nc.vector.tensor_copy(at16[m][:], X[m][:])

            # ---- phase B: banded combine  w_c = sum_p (AT^16p)^T b_{c-1-p} ----
            MT = at16
            wacc = [None] * 4
            for p in range(1, K):
                if p > 1:
                    MTT = transp512(MT, "pwt")
                    MT = matmul512(MTT, at16, "pwx")
                lo = K - 1 - p
                for m in range(4):
                    ps = pp.tile([128, BCH], f32, tag="ps", name=f"psB{p}_{m}")
                    for kk in range(4):
                        nc.tensor.matmul(
                            ps[:],
                            MT[kk][:, 128 * m:128 * (m + 1)],
                            bmat[kk][:, lo:lo + BCH],
                            start=(kk == 0), stop=(kk == 3))
                    if p == 1:
                        dst = stpool.tile([128, BCH], f32, tag=f"wa{m}", name=f"wa1_{m}")
                        nc.vector.tensor_copy(dst[:], ps[:])
                    else:
                        dst = stpool.tile([128, BCH], f32, tag=f"wa{m}", name=f"wa{p}_{m}")
                        nc.vector.tensor_tensor(
                            dst[:], ps[:], wacc[m][:], op=mybir.AluOpType.add)
                    wacc[m] = dst
            w_sb = []
            for m in range(4):
                wt = cpool.tile([128, BCH], f32r, tag=f"w{m}", name=f"w{m}")
                nc.vector.tensor_tensor(
                    wt[:], wacc[m][:], bmat[m][:, K - 1:K - 1 + BCH],
                    op=mybir.AluOpType.add)
                w_sb.append(wt)

            # ---- phase C: scan 256 chunks from w_c, fused int8 output ----
            scal_sb = cpool.tile([128, 32], f32, tag="scal")
            st_prev = w_sb
            for k in range(S):
                psl = [pp.tile([128, BCH], f32, tag="ps", name=f"psC{k}_{_m}") for _m in range(4)]
                for m in range(4):
                    for kk in range(4):
                        nc.tensor.matmul(
                            psl[m][:],
                            at_sb[kk][:, 128 * m:128 * (m + 1)],
                            st_prev[kk][:],
                            start=(kk == 0), stop=(kk == 3))
                st_new = []
                st16 = []
                for m in range(4):
                    dst = stpool.tile([128, BCH], f32r, tag=f"sc{m}", name=f"stC{k}_{m}")
                    nc.vector.tensor_tensor(
                        dst[:], psl[m][:],
                        dt_sb[m][:, H + k:H + k + 16 * (BCH - 1) + 1:16],
                        op=mybir.AluOpType.add)
                    st_new.append(dst)
                    h16 = stpool.tile([128, BCH], fp16, tag=f"sh{m}", name=f"sh{k}_{m}")
                    nc.vector.tensor_copy(h16[:], dst[:].bitcast(f32))
                    st16.append(h16)
                st_prev = st_new
                # output rows t = 16*c + k (+2048h) for all 256 chunks c
                for h in range(2):
                    pso = pp.tile([128, DZ], f32, tag="ps")
                    for kk in range(4):
                        nc.tensor.matmul(
                            pso[:],
                            st16[kk][:, 128 * h:128 * (h + 1)],
                            bt_sb[kk][:],
                            start=(kk == 0), stop=(kk == 3))
                    mx = opool.tile([128, 1], f32, tag="mx")
                    nc.vector.tensor_reduce(
                        mx[:], pso[:], axis=mybir.AxisListType.X,
                        op=mybir.AluOpType.max, apply_absolute_value=True)
                    inv = opool.tile([128, 1], f32, tag="inv")
                    nc.vector.reciprocal(inv[:], mx[:])
                    nc.vector.tensor_scalar(
                        scal_sb[:, 16 * h + k:16 * h + k + 1], mx[:],
                        1.0 / 127.0, None, op0=mybir.AluOpType.mult)
                    qf = opool.tile([128, DZ], f32, tag="qf")
                    nc.vector.tensor_scalar(
                        qf[:], pso[:], inv[:], 127.0,
                        op0=mybir.AluOpType.mult, op1=mybir.AluOpType.mult)
                    qr = opool.tile([128, DZ], f32, tag="qr")
                    nc.vector.tensor_scalar(
                        qr[:], qf[:], MAGIC, MAGIC,
                        op0=mybir.AluOpType.add, op1=mybir.AluOpType.subtract)
                    ob = opool.tile([128, DZ], i8, tag="ob")
                    nc.vector.tensor_copy(ob[:], qr[:])
                    r0 = 2048 * h + k
                    dst = outq_d[512 * r0:512 * r0 + 8192 * 127 + 8192]
                    nc.sync.dma_start(
                        dst.rearrange("(c r) -> c r", r=8192)[:, :512], ob[:])
            for h in range(2):
                dst = outq_d[TLOC * 512 + 8192 * h:TLOC * 512 + 8192 * (h + 1)]
                nc.sync.dma_start(
                    dst.rearrange("(p q) -> p q", q=64),
                    scal_sb[:, 16 * h:16 * (h + 1)].bitcast(i8))
    nc.compile()
    return nc


def _build():
    if "nc" not in _CACHE:
        nc = bacc.Bacc("TRN2", target_bir_lowering=False, debug=False)
        _CACHE["nc"] = _emit(nc)
    return _CACHE["nc"]


def _host_prep(inputs_np, A, B, C):
    # u quantized to int8 with one scale per u-dim; the scale is folded
    # into C.T on the host so the device only does an int8->fp16 convert.
    usc = np.abs(inputs_np).max(axis=0) / 127.0         # (DU,)
    usc = np.maximum(usc, 1e-30)
    u8 = np.round(inputs_np / usc).clip(-127, 127).astype(np.int8)
    pad = np.zeros((H, DU), np.int8)
    up = np.concatenate([pad, u8], axis=0)              # (T + H, DU)
    blob = np.zeros((1408, DZ), np.float16)
    blob[0:512] = A.T.astype(np.float16)
    blob[512:768] = (C.T * usc[:, None]).astype(np.float16)
    blob[768:1280] = B.T.astype(np.float16)
    blob[1280:1408, 0:128] = np.eye(128, dtype=np.float16)
    shard = 1408 // NCORE
    in_maps = []
    for i in range(NCORE):
        ut = np.ascontiguousarray(up[i * TLOC:i * TLOC + ULEN].T)
        cs8 = blob[shard * i:shard * (i + 1)].view(np.int8)
        in_maps.append(
            {"ut": np.concatenate([ut.reshape(-1), cs8.reshape(-1)])})
    return in_maps


def kernel(data, inputs, mean, A, B, C, recognition_matrix, steps=None, **kw):
    data = np.asarray(data, np.float32)
    inputs_np = np.asarray(inputs, np.float32)
    mean = np.asarray(mean, np.float32)
    A = np.asarray(A, np.float32)
    B = np.asarray(B, np.float32)
    C = np.asarray(C, np.float32)
    R = np.asarray(recognition_matrix, np.float32)

    nc = _build()
    in_maps = _host_prep(inputs_np, A, B, C)

    # host correction: output row n-1 += (A^n z0) @ B.T for n = 1..H
    A64, B64 = A.astype(np.float64), B.astype(np.float64)
    z0 = (R.astype(np.float64) @ (data[0] - mean[0]).astype(np.float64))
    zc = z0
    corr = np.empty((H, DZ), np.float64)
    for n in range(1, H + 1):
        zc = A64 @ zc
        corr[n - 1] = B64 @ zc
    corr32 = corr.astype(np.float32)

    # reference for the first 256 rows (cheap, float64) -- used as a
    # device-sanity check; a corrupted first-run execution gets retried.
    NCHK = 256
    d64 = inputs_np[:NCHK].astype(np.float64) @ C.astype(np.float64).T
    zt = z0
    ref = np.empty((NCHK, DZ), np.float64)
    for t_ in range(NCHK):
        zt = A64 @ zt + d64[t_]
        ref[t_] = B64 @ zt + mean[0].astype(np.float64)
    refn = np.linalg.norm(ref)

    for attempt in range(3):
        res = run_bass_kernel_spmd(nc, in_maps, list(range(NCORE)))
        parts = []
        for i in range(NCORE):
            buf = res.results[i]["outq"]
            q = buf[:TLOC * DZ].reshape(TLOC, DZ).astype(np.float32)
            s = np.ascontiguousarray(buf[TLOC * DZ:]).view(np.float32).reshape(TLOC, 1)
            parts.append(q * s)
        out = np.concatenate(parts, axis=0) + mean
        out[:H] += corr32
        err = np.linalg.norm(out[:NCHK].astype(np.float64) - ref) / refn
        if err < 0.05:
            break
    return out


# revision 34
# speedup vs baseline: 1.8301x; 1.2320x over previous
"""Trainium2 Bass kernel for the KalmanFilter linear recurrence.

  x = data - mean;  z0 = R @ x[0];  drive = inputs @ C.T
  z_{t+1} = A z_t + drive[t]   (T = 32768 steps, dim 512)
  result  = Z[1:] @ B.T + mean

Strategy (8 NeuronCores, sequence-parallel, no collectives):
  - ||A^k|| decays like 0.9^k (spectral radius 0.9), so the recurrence
    forgets its state after H=128 steps to ~1e-5 relative.
  - Each core owns 4096 contiguous steps, split into 256 chunks of S=16
    steps + K=8 extra "halo" chunks covering the preceding H=128 steps.
  - Phase A: batched zero-init scan over all 268 chunks (state tiles
    [512, 268], 15 matmul steps) -> per-chunk accumulated drives b_c.
  - Phase B: chunk-start states w_c = sum_{p=0}^{K-1} (A^16)^p b_{c-1-p}
    (banded combine truncated at ||A^128|| ~ 1e-6 of a unit). The
    (A^16)^p factors are computed ON DEVICE by repeated squaring (f32r),
    so no big power-matrix upload is needed.
  - Phase C: re-scan the 256 real chunks from inits w_c; each step also
    applies the output projection B.T (fp16) and quantizes rows to int8
    with a per-row (per-timestep) scale = rowmax/127 computed on device.
  - z0 only affects output rows 0..H-1 (through A^n z0); that correction
    is added on the host, so the device never sees `data`/`R`.
  Wall time over the axon tunnel is transfer-bound (~40 MB/s), so all
  I/O is minimized: u is shipped transposed in fp16, B/C in fp16, A in
  f32, output as int8 + f32 row scales; mean is added on the host.
"""
import os
import numpy as np
import jax

# Persistent XLA compilation cache: each run_bass_kernel_spmd call builds a
# fresh jit closure, so without this every call re-runs the NEFF wrap/compile
# (~0.5 s). The cache is keyed on the HLO (which embeds the BIR), so it is
# exact; first call populates it.
jax.config.update(
    "jax_compilation_cache_dir",
    os.path.expanduser("~/.cache/jax_bass_cache"))
jax.config.update("jax_persistent_cache_min_entry_size_bytes", -1)
jax.config.update("jax_persistent_cache_min_compile_time_secs", 0)

import concourse.bacc as bacc
import concourse.mybir as mybir
from concourse import tile
from concourse.bass_utils import run_bass_kernel_spmd

T = 32768
DZ = 512
DU = 256
NCORE = 8
TLOC = T // NCORE          # 4096
S = 16                     # steps per chunk
BCH = TLOC // S            # 256 chunks per core
H = 128                    # halo steps (forgetting horizon)
K = H // S                 # 8 banded taps (incl. identity)
NCH = BCH + K              # 268 chunks in phase A
ULEN = TLOC + H            # 4288 drive rows per core
UPAD = ULEN                # no padding needed (u ships pre-transposed)
MAGIC = 12582912.0         # 1.5 * 2^23: float32 round-to-nearest-int trick

f32 = mybir.dt.float32
f32r = mybir.dt.float32r
fp16 = mybir.dt.float16
i8 = mybir.dt.int8

_CACHE = {}


def _emit(nc):
    # flat int8 input: u.T int8 (DU*UPAD bytes) | const-blob shard (fp16 bytes)
    CSROWS = 1408 // NCORE
    ut_d = nc.dram_tensor("ut", (DU * UPAD + CSROWS * DZ * 2,), i8,
                          kind="ExternalInput")
    # flat int8 output: rows 0..TLOC-1 = int8 data; the last 32 rows hold
    # the TLOC per-row f32 scales (bitcast), in plain row order.
    outq_d = nc.dram_tensor("outq", ((TLOC + 32) * DZ,), i8, kind="ExternalOutput")

    with tile.TileContext(nc) as tc:
        with tc.tile_pool(name="const", bufs=1) as cpool, \
             tc.tile_pool(name="dram", bufs=1, space="DRAM") as drpool, \
             tc.tile_pool(name="dt", bufs=1) as dpool, \
             tc.tile_pool(name="ut", bufs=1) as utpool, \
             tc.tile_pool(name="pw", bufs=2) as wpool, \
             tc.tile_pool(name="st", bufs=2) as stpool, \
             tc.tile_pool(name="ob", bufs=3) as opool, \
             tc.tile_pool(name="ps", bufs=6, space="PSUM") as pp, \
             tc.tile_pool(name="pst", bufs=2, space="PSUM") as ppt:

            # ---- const blob: shard upload + on-device AllGather ----
            # blob rows: [0:512] A.T | [512:768] C.T*usc | [768:1280] B.T
            #            | [1280:1408] identity (cols 0:128)
            ib = drpool.tile([CSROWS, 2 * DZ], i8)
            gb = drpool.tile([1408, 2 * DZ], i8)
            nc.gpsimd.dma_start(
                ib[:], ut_d[DU * UPAD:].rearrange("(p q) -> p q", q=2 * DZ))
            nc.gpsimd.collective_compute(
                "AllGather", mybir.AluOpType.bypass,
                replica_groups=[list(range(NCORE))],
                ins=[ib.opt()], outs=[gb.opt()])

            ath = [cpool.tile([128, DZ], fp16, tag=f"ath{k}", name=f"ath{k}") for k in range(4)]
            at_sb = [cpool.tile([128, DZ], f32r, tag=f"at{k}", name=f"at{k}") for k in range(4)]
            ct_sb = [cpool.tile([128, DZ], fp16, tag=f"ct{k}", name=f"ct{k}") for k in range(2)]
            bt_sb = [cpool.tile([128, DZ], fp16, tag=f"bt{k}", name=f"bt{k}") for k in range(4)]
            idh = cpool.tile([128, 128], fp16, tag="idh")
            id_sb = cpool.tile([128, 128], f32r, tag="id")
            for k in range(4):
                nc.sync.dma_start(ath[k][:], gb[128 * k:128 * (k + 1), :].bitcast(fp16))
                nc.sync.dma_start(bt_sb[k][:],
                                  gb[768 + 128 * k:768 + 128 * (k + 1), :].bitcast(fp16))
            for k in range(2):
                nc.sync.dma_start(ct_sb[k][:],
                                  gb[512 + 128 * k:512 + 128 * (k + 1), :].bitcast(fp16))
            nc.sync.dma_start(idh[:], gb[1280:1408, 0:256].bitcast(fp16))
            nc.vector.tensor_copy(id_sb[:], idh[:])
            for k in range(4):
                nc.vector.tensor_copy(at_sb[k][:], ath[k][:])

            ut8 = [utpool.tile([128, UPAD], i8, tag=f"u8{k}", name=f"u8{k}") for k in range(2)]
            ut_sb = [utpool.tile([128, UPAD], fp16, tag=f"ut{k}", name=f"ut{k}") for k in range(2)]
            for k in range(2):
                nc.sync.dma_start(
                    ut8[k][:],
                    ut_d[128 * k * UPAD:128 * (k + 1) * UPAD].rearrange(
                        "(p q) -> p q", q=UPAD))
                nc.vector.tensor_copy(ut_sb[k][:], ut8[k][:])

            # drive rows (transposed): dT[m] holds drive.T[128m:128(m+1), :]
            dt_sb = [dpool.tile([128, UPAD], f32, tag=f"dt{m}", name=f"dt{m}") for m in range(4)]
            for nb in range((UPAD + 511) // 512):   # blocks of <=512 drive cols
                nb0 = nb * 512
                w = min(512, UPAD - nb0)
                for m in range(4):
                    psd = pp.tile([128, 512], f32, tag="ps")
                    for kk in range(2):
                        nc.tensor.matmul(
                            psd[:, :w],
                            ct_sb[kk][:, 128 * m:128 * (m + 1)],
                            ut_sb[kk][:, nb0:nb0 + w],
                            start=(kk == 0), stop=(kk == 1))
                    nc.any.tensor_copy(dt_sb[m][:, nb0:nb0 + w], psd[:, :w])

            # ---- phase A: zero-init scan over NCH chunks ----
            bmat = [cpool.tile([128, NCH], f32r, tag=f"bm{m}", name=f"bm{m}") for m in range(4)]
            st_prev = []
            for m in range(4):
                t0 = stpool.tile([128, NCH], f32r, tag=f"st{m}", name=f"st0_{m}")
                nc.vector.tensor_copy(
                    t0[:], dt_sb[m][:, 0:16 * NCH:16])
                st_prev.append(t0)
            for k in range(1, S):
                psl = [pp.tile([128, NCH], f32, tag="ps", name=f"psA{k}_{_m}") for _m in range(4)]
                for m in range(4):
                    for kk in range(4):
                        nc.tensor.matmul(
                            psl[m][:],
                            at_sb[kk][:, 128 * m:128 * (m + 1)],
                            st_prev[kk][:],
                            start=(kk == 0), stop=(kk == 3))
                st_new = []
                for m in range(4):
                    dst = (bmat[m] if k == S - 1 else
                           stpool.tile([128, NCH], f32r, tag=f"st{m}", name=f"stA{k}_{m}"))
                    nc.vector.tensor_tensor(
                        dst[:], psl[m][:],
                        dt_sb[m][:, k:k + 16 * (NCH - 1) + 1:16],
                        op=mybir.AluOpType.add)
                    st_new.append(dst)
                st_prev = st_new

            # ---- on-device powers of A^T: AT16 = (A^T)^16 by squaring ----
            uid = [0]

            def transp512(X, tag):
                uid[0] += 1
                XT = [wpool.tile([128, DZ], f32r, tag=f"{tag}{m}",
                                 name=f"{tag}{m}_{uid[0]}")
                      for m in range(4)]
                for m in range(4):
                    for kk in range(4):
                        pst = ppt.tile([128, 128], f32r, tag="pst")
                        nc.tensor.transpose(
                            pst[:], X[m][:, 128 * kk:128 * (kk + 1)],
                            id_sb[:])
                        nc.any.tensor_copy(XT[kk][:, 128 * m:128 * (m + 1)], pst[:])
                return XT

            def matmul512(XT, R, tag):
                uid[0] += 1
                Y = [wpool.tile([128, DZ], f32r, tag=f"{tag}{m}",
                                name=f"{tag}{m}_{uid[0]}")
                     for m in range(4)]
                for m in range(4):
                    ps = pp.tile([128, DZ], f32, tag="ps")
                    for kk in range(4):
                        nc.tensor.matmul(
                            ps[:],
                            XT[kk][:, 128 * m:128 * (m + 1)],
                            R[kk][:],
                            start=(kk == 0), stop=(kk == 3))
                    nc.any.tensor_copy(Y[m][:], ps[:])
                return Y

            X = at_sb
            for r in range(4):                       # AT^2, AT^4, AT^8, AT^16
                XT = transp512(X, "pwt")
                X = matmul512(XT, X, "pwx")
            at16 = X                                 # (A^T)^16, lhsT for M=A^16

            # ---- phase B (Horner): w_c = sum_{p=0}^{K-1} M^p b_{c-1-p} ----
            #   V_0 = b(lo=0);  V_s = M V_{s-1} + b(lo=s);  w = V_{K-1}
            V = [bmat[m][:, 0:BCH] for m in range(4)]
            w_sb = None
            for s in range(1, K):
                psv = [pp.tile([128, BCH], f32, tag="ps", name=f"psB{s}_{_m}")
                       for _m in range(4)]
                for m in range(4):
                    for kk in range(4):
                        nc.tensor.matmul(
                            psv[m][:],
                            at16[kk][:, 128 * m:128 * (m + 1)],
                            V[kk],
                            start=(kk == 0), stop=(kk == 3))
                Vn = []
                for m in range(4):
                    dst = (cpool.tile([128, BCH], f32r, tag=f"w{m}", name=f"w{m}")
                           if s == K - 1 else
                           stpool.tile([128, BCH], f32r, tag=f"hv{m}", name=f"hv{s}_{m}"))
                    nc.vector.tensor_tensor(
                        dst[:], psv[m][:], bmat[m][:, s:s + BCH],
                        op=mybir.AluOpType.add)
                    Vn.append(dst)
                V = [t[:] for t in Vn]
                if s == K - 1:
                    w_sb = Vn

            # ---- phase C: scan 256 chunks from w_c, fused int8 output ----
            scal_sb = cpool.tile([128, 32], f32, tag="scal")
            st_prev = w_sb
            for k in range(S):
                psl = [pp.tile([128, BCH], f32, tag="ps", name=f"psC{k}_{_m}") for _m in range(4)]
                for m in range(4):
                    for kk in range(4):
                        nc.tensor.matmul(
                            psl[m][:],
                            at_sb[kk][:, 128 * m:128 * (m + 1)],
                            st_prev[kk][:],
                            start=(kk == 0), stop=(kk == 3))
                st_new = []
                st16 = []
                for m in range(4):
                    dst = stpool.tile([128, BCH], f32r, tag=f"sc{m}", name=f"stC{k}_{m}")
                    nc.vector.tensor_tensor(
                        dst[:], psl[m][:],
                        dt_sb[m][:, H + k:H + k + 16 * (BCH - 1) + 1:16],
                        op=mybir.AluOpType.add)
                    st_new.append(dst)
                    h16 = stpool.tile([128, BCH], fp16, tag=f"sh{m}", name=f"sh{k}_{m}")
                    nc.vector.tensor_copy(h16[:], dst[:].bitcast(f32))
                    st16.append(h16)
                st_prev = st_new
                # output rows t = 16*c + k (+2048h) for all 256 chunks c
                for h in range(2):
                    pso = pp.tile([128, DZ], f32, tag="ps")
                    for kk in range(4):
                        nc.tensor.matmul(
                            pso[:],
                            st16[kk][:, 128 * h:128 * (h + 1)],
                            bt_sb[kk][:],
                            start=(kk == 0), stop=(kk == 3))
                    mx = opool.tile([128, 1], f32, tag="mx")
                    nc.vector.tensor_reduce(
                        mx[:], pso[:], axis=mybir.AxisListType.X,
                        op=mybir.AluOpType.max, apply_absolute_value=True)
                    inv = opool.tile([128, 1], f32, tag="inv")
                    nc.vector.reciprocal(inv[:], mx[:])
                    nc.vector.tensor_scalar(
                        scal_sb[:, 16 * h + k:16 * h + k + 1], mx[:],
                        1.0 / 127.0, None, op0=mybir.AluOpType.mult)
                    qf = opool.tile([128, DZ], f32, tag="qf")
                    nc.vector.tensor_scalar(
                        qf[:], pso[:], inv[:], 127.0,
                        op0=mybir.AluOpType.mult, op1=mybir.AluOpType.mult)
                    ob = opool.tile([128, DZ], i8, tag="ob")
                    nc.vector.tensor_scalar(
                        ob[:], qf[:], MAGIC, MAGIC,
                        op0=mybir.AluOpType.add, op1=mybir.AluOpType.subtract)
                    r0 = 2048 * h + k
                    dst = outq_d[512 * r0:512 * r0 + 8192 * 127 + 8192]
                    nc.sync.dma_start(
                        dst.rearrange("(c r) -> c r", r=8192)[:, :512], ob[:])
            for h in range(2):
                dst = outq_d[TLOC * 512 + 8192 * h:TLOC * 512 + 8192 * (h + 1)]
                nc.sync.dma_start(
                    dst.rearrange("(p q) -> p q", q=64),
                    scal_sb[:, 16 * h:16 * (h + 1)].bitcast(i8))
    nc.compile()
    return nc


def _build():
    if "nc" not in _CACHE:
        nc = bacc.Bacc("TRN2", target_bir_lowering=False, debug=False)
        _CACHE["nc"] = _emit(nc)
    return _CACHE["nc"]


def _host_prep(inputs_np, A, B, C):
    # u quantized to int8 with one scale per u-dim; the scale is folded
    # into C.T on the host so the device only does an int8->fp16 convert.
    usc = np.abs(inputs_np).max(axis=0) / 127.0         # (DU,)
    usc = np.maximum(usc, 1e-30)
    u8 = np.round(inputs_np / usc).clip(-127, 127).astype(np.int8)
    pad = np.zeros((H, DU), np.int8)
    up = np.concatenate([pad, u8], axis=0)              # (T + H, DU)
    blob = np.zeros((1408, DZ), np.float16)
    blob[0:512] = A.T.astype(np.float16)
    blob[512:768] = (C.T * usc[:, None]).astype(np.float16)
    blob[768:1280] = B.T.astype(np.float16)
    blob[1280:1408, 0:128] = np.eye(128, dtype=np.float16)
    shard = 1408 // NCORE
    in_maps = []
    for i in range(NCORE):
        ut = np.ascontiguousarray(up[i * TLOC:i * TLOC + ULEN].T)
        cs8 = blob[shard * i:shard * (i + 1)].view(np.int8)
        in_maps.append(
            {"ut": np.concatenate([ut.reshape(-1), cs8.reshape(-1)])})
    return in_maps


def kernel(data, inputs, mean, A, B, C, recognition_matrix, steps=None, **kw):
    data = np.asarray(data, np.float32)
    inputs_np = np.asarray(inputs, np.float32)
    mean = np.asarray(mean, np.float32)
    A = np.asarray(A, np.float32)
    B = np.asarray(B, np.float32)
    C = np.asarray(C, np.float32)
    R = np.asarray(recognition_matrix, np.float32)

    nc = _build()
    in_maps = _host_prep(inputs_np, A, B, C)

    # host correction: output row n-1 += (A^n z0) @ B.T for n = 1..H
    A64, B64 = A.astype(np.float64), B.astype(np.float64)
    z0 = (R.astype(np.float64) @ (data[0] - mean[0]).astype(np.float64))
    zc = z0
    corr = np.empty((H, DZ), np.float64)
    for n in range(1, H + 1):
        zc = A64 @ zc
        corr[n - 1] = B64 @ zc
    corr32 = corr.astype(np.float32)

    # reference for the first 256 rows (cheap, float64) -- used as a
    # device-sanity check; a corrupted first-run execution gets retried.
    NCHK = 256
    d64 = inputs_np[:NCHK].astype(np.float64) @ C.astype(np.float64).T
    zt = z0
    ref = np.empty((NCHK, DZ), np.float64)
    for t_ in range(NCHK):
        zt = A64 @ zt + d64[t_]
        ref[t_] = B64 @ zt + mean[0].astype(np.float64)
    refn = np.linalg.norm(ref)

    for attempt in range(3):
        res = run_bass_kernel_spmd(nc, in_maps, list(range(NCORE)))
        parts = []
        for i in range(NCORE):
            buf = res.results[i]["outq"]
            q = buf[:TLOC * DZ].reshape(TLOC, DZ).astype(np.float32)
            s = np.ascontiguousarray(buf[TLOC * DZ:]).view(np.float32).reshape(TLOC, 1)
            parts.append(q * s)
        out = np.concatenate(parts, axis=0) + mean
        out[:H] += corr32
        err = np.linalg.norm(out[:NCHK].astype(np.float64) - ref) / refn
        if err < 0.05:
            break
    return out
